# revision 49
# baseline (speedup 1.0000x reference)
"""Bass/Trainium2 kernel for a single-head causal decoder attention head.

Reference computation (fp32):
    k = x @ Wk; q = x @ Wq; v = x @ Wv            # [B,T,H]
    att = softmax(causal(q k^T / sqrt(H)))        # [B,T,T]
    out = att @ v                                 # [B,T,H]
with B=4, T=4096, C=1024, H=128.

Sharding: 8 cores = 4 batches x 2 query-interleave lanes (j in {0,1}).
Core (b, j) handles q-blocks {(2i+j)*512 : i in 0..3}.  The host hands
each core a *permuted, group-contiguous* x^T whose 512-col groups are
[own-lane blocks | other-lane blocks], so every core runs one identical
instruction stream (SPMD).  Causality reduces to a lane-independent
triangular mask on the own-section diagonal block plus a per-lane
all-0/all-1 scalar on the final other-section chunks.

Key scheduling choices (v2):
  - x is packed group-contiguous on the host (8 KB/partition descriptor
    runs) and fully prefetched into SBUF across the sync/scalar/gpsimd
    DMA rings; group 0 streams per-chunk so the first K matmul starts
    ~1.5 us after the preamble.
  - proj/att interleave is dependency-minimal: proj0, proj4, att0,
    proj1, proj5, att1, ... (att(i) only needs proj groups {0..i, 4..4+i}).
  - softmax denominators: groups 0..2 fold all the way to one tile
    (extra tree adds on gpsimd) -> single sums-matmul; the last group
    keeps the progressive quad scheme so its epilogue chain stays short.
  - pad-masking and pair/quad folds run on gpsimd, keeping DVE free for
    block folds / diagonal masks / PSUM copies.
"""

import sys

sys.path.insert(0, "/opt/trn_rl_repo")

import numpy as np
import ml_dtypes

import concourse.mybir as mybir
import concourse.tile as tile
from concourse import bacc
from concourse.alu_op_type import AluOpType
from concourse.masks import make_identity
from concourse.bass_utils import run_bass_kernel_spmd

B, T, C, H = 4, 4096, 1024, 128
NCORES = 8
QG = 512                      # q-group width
NG = 4                        # q-groups per core
CB = C // 128                 # 8 contraction chunks
TGRP = T // QG                # 8 column groups of x^T
SCALE = float(H) ** -0.5
GPSIMD_FOLDS = False          # gpsimd elementwise is Q7 software: ~25x slower
                              # than DVE ([128,1024] tensor_scalar ~13us) - never route
                              # fold/mask work there

BF16 = mybir.dt.bfloat16
F32 = mybir.dt.float32
NPBF16 = ml_dtypes.bfloat16


def _build_program():
    nc = bacc.Bacc("TRN2", target_bir_lowering=False, debug=False)

    # group-contiguous packed x^T: group g at cols [g*CB*QG, (g+1)*CB*QG),
    # laid out chunk-major: [p, g, c, t]
    xt = nc.dram_tensor("xt", [128, TGRP * CB * QG], BF16, kind="ExternalInput").ap()
    wk = nc.dram_tensor("wk", [128, CB * H], BF16, kind="ExternalInput").ap()
    wq = nc.dram_tensor("wq", [128, CB * H], BF16, kind="ExternalInput").ap()
    wv = nc.dram_tensor("wv", [128, CB * H], BF16, kind="ExternalInput").ap()
    msk = nc.dram_tensor("msk", [128, 4 * QG], BF16, kind="ExternalInput").ap()
    pad = nc.dram_tensor("pad", [128, 1], F32, kind="ExternalInput").ap()
    outT = nc.dram_tensor("outT", [H, NG * QG], BF16, kind="ExternalOutput").ap()

    GW = CB * QG              # 4096 cols per x group

    with tile.TileContext(nc) as tc:
        with (
            tc.tile_pool(name="const", bufs=1) as constp,
            tc.tile_pool(name="kvq", bufs=1) as kvqp,
            tc.tile_pool(name="vtb", bufs=2) as vtbp,
            tc.tile_pool(name="attb", bufs=4) as attp,
            tc.tile_pool(name="foldb", bufs=6) as foldp,
            tc.tile_pool(name="epi", bufs=2) as epip,
            tc.tile_pool(name="pp", bufs=2, space="PSUM") as ppool,
            tc.tile_pool(name="ap", bufs=1, space="PSUM") as apool,
        ):
            # --- persistent SBUF tensors ---
            wks = constp.tile([128, CB * H], BF16, tag="wks")
            wqs = constp.tile([128, CB * H], BF16, tag="wqs")
            wvs = constp.tile([128, CB * H], BF16, tag="wvs")
            masks = constp.tile([128, 4 * QG], BF16, tag="masks")
            padv = constp.tile([128, 1], F32, tag="padv")
            ident = constp.tile([128, 128], BF16, tag="ident")
            XA = constp.tile([128, TGRP * GW], BF16, tag="XA")

            KT = kvqp.tile([128, T], BF16, tag="KT")
            VV = kvqp.tile([128, T], BF16, tag="VV")
            QT = kvqp.tile([128, NG * QG], BF16, tag="QT")
            ones = kvqp.tile([128, 128], BF16, tag="ones")

            # --- input DMA prefetch ---------------------------------------
            # All 16 SDMA engines saturate for ~22us moving x (8 MB), and
            # they round-robin fairly across rings - so both HWDGE rings
            # (scalar, sync) are fed in strict consumption-priority order,
            # interleaved so the two FIFO heads track the same priority
            # frontier: wk, g0, wv|wq, g4, masks, g1, g5, g2, g6, g3, g7.
            # Groups after g0 are split in halves across the rings.
            # both rings lead with half the K weights, then alternate x
            # chunks, so the first K matmul's dependencies land in ~1us
            nc.sync.dma_start(wks[:, :CB * H // 2], wk[:, :CB * H // 2])
            nc.scalar.dma_start(wks[:, CB * H // 2:], wk[:, CB * H // 2:])
            # last two group-0 chunks ride the otherwise-idle gpsimd ring so
            # the two HWDGE rings deliver c0-c5 and the weights sooner
            for ci in (0, 2, 4):
                nc.sync.dma_start(
                    XA[:, ci * QG:(ci + 1) * QG], xt[:, ci * QG:(ci + 1) * QG]
                )
            for ci in (1, 3, 5):
                nc.scalar.dma_start(
                    XA[:, ci * QG:(ci + 1) * QG], xt[:, ci * QG:(ci + 1) * QG]
                )
            for ci in (6, 7):
                nc.gpsimd.dma_start(
                    XA[:, ci * QG:(ci + 1) * QG], xt[:, ci * QG:(ci + 1) * QG]
                )
            HG = GW // 2
            def xgrp(g, ring_a, ring_b):
                ring_a.dma_start(
                    XA[:, g * GW:g * GW + HG], xt[:, g * GW:g * GW + HG]
                )
                ring_b.dma_start(
                    XA[:, g * GW + HG:(g + 1) * GW], xt[:, g * GW + HG:(g + 1) * GW]
                )
            nc.sync.dma_start(wvs, wv)
            nc.scalar.dma_start(wqs, wq)
            xgrp(4, nc.sync, nc.scalar)
            nc.sync.dma_start(masks, msk)
            nc.scalar.dma_start(padv, pad)
            for g in (1, 5, 2, 6, 3, 7):
                xgrp(g, nc.scalar, nc.sync)
            nc.vector.memset(ones, 1.0)
            make_identity(nc, ident)

            pend_tr = []          # deferred (vtt, tg) transpose work

            def do_transposes():
                while pend_tr:
                    vtt, tg = pend_tr.pop(0)
                    tps = ppool.tile([128, QG], BF16, tag="pps")
                    for tb in range(QG // 128):
                        nc.tensor.transpose(
                            tps[:, tb * 128:(tb + 1) * 128],
                            vtt[:, tb * 128:(tb + 1) * 128],
                            ident,
                        )
                    nc.vector.tensor_copy(VV[:, tg * QG:(tg + 1) * QG], tps)

            def proj_gen(tg, with_q, interleave=False):
                """Projection emitted as a generator: the K and V matmul
                loops yield per-MM so the attention tile loop can feed them
                into its PE slack; the DVE copies / transposes / Q stay in
                the drain (attention's DVE and ACT are the gated engines).
                interleave=True pairs K and V per chunk so the DMA-paced
                startup groups do two matmuls per arriving x chunk."""
                xg = XA[:, tg * GW:(tg + 1) * GW]
                kps = ppool.tile([128, QG], F32, tag="pps")
                if interleave:
                    vps = ppool.tile([128, QG], F32, tag="pps")
                    for ci in range(CB):
                        nc.tensor.matmul(
                            kps,
                            lhsT=wks[:, ci * H:(ci + 1) * H],
                            rhs=xg[:, ci * QG:(ci + 1) * QG],
                            start=(ci == 0),
                            stop=(ci == CB - 1),
                        )
                        nc.tensor.matmul(
                            vps,
                            lhsT=wvs[:, ci * H:(ci + 1) * H],
                            rhs=xg[:, ci * QG:(ci + 1) * QG],
                            start=(ci == 0),
                            stop=(ci == CB - 1),
                        )
                        yield
                else:
                    for ci in range(CB):
                        nc.tensor.matmul(
                            kps,
                            lhsT=wks[:, ci * H:(ci + 1) * H],
                            rhs=xg[:, ci * QG:(ci + 1) * QG],
                            start=(ci == 0),
                            stop=(ci == CB - 1),
                        )
                        yield
                    vps = ppool.tile([128, QG], F32, tag="pps")
                    for ci in range(CB):
                        nc.tensor.matmul(
                            vps,
                            lhsT=wvs[:, ci * H:(ci + 1) * H],
                            rhs=xg[:, ci * QG:(ci + 1) * QG],
                            start=(ci == 0),
                            stop=(ci == CB - 1),
                        )
                        yield
                nc.vector.tensor_copy(KT[:, tg * QG:(tg + 1) * QG], kps)
                do_transposes()    # previous group's V transposes (inputs ready)
                vtt = vtbp.tile([128, QG], BF16, tag="vtt")
                nc.vector.tensor_copy(vtt, vps)
                pend_tr.append((vtt, tg))
                if with_q:
                    qps = ppool.tile([128, QG], F32, tag="pps")
                    for ci in range(CB):
                        nc.tensor.matmul(
                            qps,
                            lhsT=wqs[:, ci * H:(ci + 1) * H],
                            rhs=xg[:, ci * QG:(ci + 1) * QG],
                            start=(ci == 0),
                            stop=(ci == CB - 1),
                        )
                    nc.vector.tensor_copy(QT[:, tg * QG:(tg + 1) * QG], qps)

            def qgen(tg):
                """Q projection alone - hoisted ahead of its group's K/V so
                the following attention group can start before K/V land."""
                xg = XA[:, tg * GW:(tg + 1) * GW]
                qps = ppool.tile([128, QG], F32, tag="pps")
                for ci in range(CB):
                    nc.tensor.matmul(
                        qps,
                        lhsT=wqs[:, ci * H:(ci + 1) * H],
                        rhs=xg[:, ci * QG:(ci + 1) * QG],
                        start=(ci == 0),
                        stop=(ci == CB - 1),
                    )
                    yield
                nc.vector.tensor_copy(QT[:, tg * QG:(tg + 1) * QG], qps)

            def kvgen(tg):
                """K/V projection without Q - the JIT-streamed remainder."""
                xg = XA[:, tg * GW:(tg + 1) * GW]
                kps = ppool.tile([128, QG], F32, tag="pps")
                for ci in range(CB):
                    nc.tensor.matmul(
                        kps,
                        lhsT=wks[:, ci * H:(ci + 1) * H],
                        rhs=xg[:, ci * QG:(ci + 1) * QG],
                        start=(ci == 0),
                        stop=(ci == CB - 1),
                    )
                    yield
                vps = ppool.tile([128, QG], F32, tag="pps")
                for ci in range(CB):
                    nc.tensor.matmul(
                        vps,
                        lhsT=wvs[:, ci * H:(ci + 1) * H],
                        rhs=xg[:, ci * QG:(ci + 1) * QG],
                        start=(ci == 0),
                        stop=(ci == CB - 1),
                    )
                    yield
                nc.vector.tensor_copy(KT[:, tg * QG:(tg + 1) * QG], kps)
                do_transposes()
                vtt = vtbp.tile([128, QG], BF16, tag="vtt")
                nc.vector.tensor_copy(vtt, vps)
                pend_tr.append((vtt, tg))

            feeders = []
            jit_drains = {}       # att tile index -> generator to drain first

            def feed(n):
                while n > 0 and feeders:
                    try:
                        next(feeders[0])
                        n -= 1
                    except StopIteration:
                        feeders.pop(0)

            def drain_gen(g):
                # exhaust the FIFO up to and including generator g
                while g in feeders:
                    try:
                        next(feeders[0])
                    except StopIteration:
                        feeders.pop(0)

            def proj(tg, with_q):
                for _ in proj_gen(tg, with_q):
                    pass

            def att(i):
                do_transposes()    # flush V transposes the group reads
                qg = QT[:, i * QG:(i + 1) * QG]
                # consecutive att groups are separated by two proj groups,
                # so the previous group's epilogue has long released these
                otps = apool.tile([128, QG], F32, tag="otps", bufs=1)
                smps = apool.tile([128, QG], F32, tag="smps", bufs=1)
                ntiles = 2 * (i + 1)
                # tiles: chunk base; mask kind (None | diag-offset | 'pad')
                tiles = []
                for sec in range(2):
                    for tp in range(ntiles):
                        mt = tp - (ntiles - 2)
                        if mt < 0:
                            mk = None
                        elif sec == 0:
                            mk = mt * 2 * QG
                        else:
                            mk = "pad"
                        tiles.append((16 * sec + 2 * tp, mk))
                ntot = len(tiles)
                sps_t = [None] * ntot
                pt_t = [None] * ntot
                fold_t = [None] * ntot

                HQ = QG // 2

                def emit_s(t):
                    c0, mk = tiles[t]
                    # upper diagonal tile (kv offsets 256..511): fully masked
                    # for q < 256, so only compute the upper q-half; the mask
                    # multiply zeroes the stale lower half before use.  NOT
                    # for group 0: its tile 1 is the psum slot's first-ever
                    # use, and exp(uninitialized psum) can be inf -> inf*0
                    # mask -> NaN.  Later groups reuse slots already holding
                    # finite S values, so their stale halves exp to finite.
                    sh = HQ if (mk == 2 * QG and i > 0) else 0
                    sps = apool.tile([128, 2 * QG], F32, tag="sps", bufs=2)
                    for h in range(2):
                        nc.tensor.matmul(
                            sps[:, h * QG + sh:(h + 1) * QG],
                            lhsT=KT[:, (c0 + h) * 128:(c0 + h + 1) * 128],
                            rhs=qg[:, sh:],
                            start=True,
                            stop=True,
                        )
                    sps_t[t] = sps

                def emit_exp_mask_fold(t):
                    _, mk = tiles[t]
                    pt = attp.tile([128, 2 * QG], BF16, tag="pt")
                    # pad tiles: fold the per-lane kill into the exp bias
                    # (lane 0: -1e9 -> exp underflows to exact 0; lane 1: 0)
                    nc.scalar.activation(
                        pt, sps_t[t], mybir.ActivationFunctionType.Exp,
                        scale=SCALE,
                        bias=padv if mk == "pad" else 0.0,
                    )
                    sps_t[t] = None
                    if mk is not None and mk != "pad":
                        nc.vector.tensor_tensor(
                            pt, pt, masks[:, mk:mk + 2 * QG], op=AluOpType.mult
                        )
                    pt_t[t] = pt
                    if last and t == ntot - 1:
                        return    # final tile sums straight from pt halves
                    fold = foldp.tile([128, QG], BF16, tag="fold")
                    nc.vector.tensor_tensor(
                        fold, pt[:, 0:QG], pt[:, QG:2 * QG], op=AluOpType.add
                    )
                    fold_t[t] = fold

                def emit_pv(t):
                    c0, mk = tiles[t]
                    sh = HQ if (mk == 2 * QG and i > 0) else 0  # match emit_s
                    for h in range(2):
                        c = c0 + h
                        nc.tensor.matmul(
                            otps[:, sh:],
                            lhsT=VV[:, c * 128:(c + 1) * 128],
                            rhs=pt_t[t][:, h * QG + sh:(h + 1) * QG],
                            start=(t == 0 and h == 0),
                            stop=(t == ntot - 1 and h == 1),
                        )
                    pt_t[t] = None

                feng = nc.gpsimd if GPSIMD_FOLDS else nc.vector

                ffs = []        # pair-folds awaiting a quad partner
                qmm = []        # quad-folds awaiting their sums matmul
                nsum = [0]
                deep = i < NG - 1      # groups 0..2: fold to one sums-matmul
                last = i == NG - 1     # final group: short epilogue chain
                NSUM = 1 if deep else (ntot // 4 - 1) + 4

                def emit_pair(t):
                    # pair-fold tiles (t-1, t); every 2nd pair quad-folds
                    ff = foldp.tile([128, QG], BF16, tag="ffold", bufs=4)
                    feng.tensor_tensor(
                        ff, fold_t[t - 1], fold_t[t], op=AluOpType.add
                    )
                    fold_t[t - 1] = fold_t[t] = None
                    ffs.append(ff)
                    if len(ffs) == 2:
                        fff = foldp.tile([128, QG], BF16, tag="fff", bufs=3)
                        feng.tensor_tensor(
                            fff, ffs[0], ffs[1], op=AluOpType.add
                        )
                        ffs.clear()
                        qmm.append(fff)

                def flush_sums(final=False):
                    if deep:
                        # fold quads into one running tile; single sums-matmul
                        while len(qmm) >= 2:
                            a = qmm.pop(0)
                            b = qmm.pop(0)
                            acc = foldp.tile([128, QG], BF16, tag="fff", bufs=3)
                            feng.tensor_tensor(acc, a, b, op=AluOpType.add)
                            qmm.insert(0, acc)
                        if final:
                            assert len(qmm) == 1
                            fff = qmm.pop(0)
                            nc.tensor.matmul(
                                smps, lhsT=ones, rhs=fff, start=True, stop=True,
                            )
                        return
                    while qmm:
                        fff = qmm.pop(0)
                        q = nsum[0]
                        nsum[0] += 1
                        nc.tensor.matmul(
                            smps, lhsT=ones, rhs=fff,
                            start=(q == 0), stop=(q == NSUM - 1),
                        )

                emit_s(0)
                emit_exp_mask_fold(0)
                for t in range(ntot):
                    if (t + 1) in jit_drains:
                        # JIT-streamed K/V must land before the next tile's
                        # S/PV read it; flush the pending V transposes too
                        drain_gen(jit_drains.pop(t + 1))
                        do_transposes()
                    if t + 1 < ntot:
                        emit_s(t + 1)
                        emit_exp_mask_fold(t + 1)
                    if last and t == ntot - 1:
                        # promote pair(n-4,n-3) and fold(n-2) straight to
                        # sums-matmuls before the last PV, so the epilogue
                        # chain after exp(n-1) is just sums -> recip
                        ff = foldp.tile([128, QG], BF16, tag="ffold", bufs=4)
                        nc.vector.tensor_tensor(
                            ff, fold_t[t - 3], fold_t[t - 2], op=AluOpType.add
                        )
                        fold_t[t - 3] = fold_t[t - 2] = None
                        qmm.append(ff)
                        qmm.append(fold_t[t - 1])
                        fold_t[t - 1] = None
                        # the final tile's unfolded pt halves go straight to
                        # sums-matmuls BEFORE its PV, so the reciprocals run
                        # during the last PV instead of after it
                        qmm.append(pt_t[t][:, 0:QG])
                        qmm.append(pt_t[t][:, QG:2 * QG])
                        flush_sums()
                    emit_pv(t)
                    feed(1)    # one deferred proj MM into this tile's PE slack
                    if t % 2 == 1 and t >= 3 and not (last and t == ntot - 1):
                        flush_sums()          # quads trail two more tiles
                        emit_pair(t - 2)      # pairs trail two tiles
                if not last:
                    emit_pair(ntot - 1)
                    flush_sums(final=True)
                HQ = QG // 2
                for hh in range(2):
                    sl = slice(hh * HQ, (hh + 1) * HQ)
                    rb = epip.tile([128, HQ], F32, tag="rb", bufs=2)
                    nc.vector.reciprocal_approx_fast(rb, smps[:, sl])
                    ot = epip.tile([128, HQ], BF16, tag="ot", bufs=2)
                    nc.vector.tensor_tensor(ot, otps[:, sl], rb,
                                            op=AluOpType.mult)
                    dst = outT[:, i * QG + hh * HQ:i * QG + (hh + 1) * HQ]
                    # alternate rings so the two ~600ns DMA triggers run in
                    # parallel; the final group uses the vector/gpsimd rings
                    # (same-engine ordering after the mult - no sem hop)
                    (nc.sync if hh == 0 else nc.scalar).dma_start(dst, ot)

            # just-in-time projection streaming: att(i) only needs Q(i) up
            # front; its own K/V aren't read until the diagonal tiles (2i)
            # and the other-lane K/V until tile 2(i+1)+2i, so groups 2,3
            # stream their K/V into the preceding atts' exp-gated PE slack
            proj(0, with_q=True)
            proj(4, with_q=False)
            qg1, kv1, kv5 = qgen(1), kvgen(1), kvgen(5)
            feeders[:] = [qg1, kv1, kv5]
            att(0)
            drain_gen(kv1)           # att1 reads K1/V1 at its diag tile 2
            jit_drains[6] = kv5      # att1 other tail
            qg2, kv2, kv6 = qgen(2), kvgen(2), kvgen(6)
            feeders.extend([qg2, kv2, kv6])
            att(1)
            drain_gen(qg2)
            qg3, kv3, kv7 = qgen(3), kvgen(3), kvgen(7)
            feeders.extend([qg3, kv3, kv7])
            jit_drains[4] = kv2      # att2 own diag
            jit_drains[10] = kv6     # att2 other tail
            att(2)
            drain_gen(qg3)
            jit_drains[6] = kv3      # att3 own diag
            jit_drains[14] = kv7     # att3 other tail
            att(3)

    if not nc.is_finalized():
        nc.finalize()
    return nc


_NC_CACHE = None


def _get_program():
    global _NC_CACHE
    if _NC_CACHE is None:
        _NC_CACHE = _build_program()
    return _NC_CACHE


def _make_masks() -> np.ndarray:
    """Triangular masks [128, 2048] for the 4 chunks of the own-section
    diagonal block (chunk c masked where 128*c + kv > q), lane-independent."""
    out = np.empty((128, 4 * QG), np.float32)
    kv = np.arange(128)[:, None]
    q = np.arange(QG)[None, :]
    for c in range(4):
        out[:, c * QG:(c + 1) * QG] = (128 * c + kv <= q)
    return out.astype(NPBF16)


def _pack_weight(w: np.ndarray) -> np.ndarray:
    # [C, H] -> [128, CB*H] with chunk-major cols: out[p, c*H+h] = w[c*128+p, h]
    return np.ascontiguousarray(
        w.reshape(CB, 128, H).transpose(1, 0, 2).reshape(128, CB * H)
    ).astype(NPBF16)


def _run(inputs: dict, trace: bool = False, trace_kwargs: dict | None = None):
    x = np.asarray(inputs["x"], np.float32)
    Wk = np.asarray(inputs["Wk"], np.float32)
    Wq = np.asarray(inputs["Wq"], np.float32)
    Wv = np.asarray(inputs["Wv"], np.float32)

    nc = _get_program()

    wk16 = _pack_weight(Wk)
    wq16 = _pack_weight(Wq)
    wv16 = _pack_weight(Wv)
    msk = _make_masks()
    # exp-bias pad kill: lane 0 gets -1e9 (exp -> 0), lane 1 gets 0 (keep)
    pads = [np.full((128, 1), (j - 1) * 1e9, np.float32) for j in range(2)]

    in_maps = []
    for b in range(B):
        # [CB, 128, TGRP, QG]: xtr[c, p, blk, t] = x[b, blk*QG+t, c*128+p]
        xtr = np.ascontiguousarray(x[b].T).astype(NPBF16).reshape(
            CB, 128, TGRP, QG
        )
        for j in range(2):
            order = [2 * i + j for i in range(NG)] + [
                2 * i + 1 - j for i in range(NG)
            ]
            # pack group-contiguous chunk-major: xp[p, g, c, t]
            xp = np.ascontiguousarray(
                xtr[:, :, order, :].transpose(1, 2, 0, 3)
            ).reshape(128, TGRP * CB * QG)
            in_maps.append(
                {
                    "xt": xp,
                    "wk": wk16,
                    "wq": wq16,
                    "wv": wv16,
                    "msk": msk,
                    "pad": pads[j],
                }
            )

    res = run_bass_kernel_spmd(
        nc,
        in_maps,
        core_ids=list(range(NCORES)),
        trace=trace,
        **(trace_kwargs or {}),
    )

    out = np.empty((B, T, H), np.float32)
    for core in range(NCORES):
        b, j = divmod(core, 2)
        oT = np.asarray(res.results[core]["outT"], np.float32)  # [H, NG*QG]
        for i in range(NG):
            g = (2 * i + j) * QG
            out[b, g:g + QG, :] = oT[:, i * QG:(i + 1) * QG].T
    return out, res


def kernel(**inputs) -> np.ndarray:
    out, _ = _run(inputs, trace=False)
    return out


# revision 50
# speedup vs baseline: 1.1875x; 1.1875x over previous
"""Bass/Trainium2 kernel for a single-head causal decoder attention head.

Reference computation (fp32):
    k = x @ Wk; q = x @ Wq; v = x @ Wv            # [B,T,H]
    att = softmax(causal(q k^T / sqrt(H)))        # [B,T,T]
    out = att @ v                                 # [B,T,H]
with B=4, T=4096, C=1024, H=128.

Sharding: 8 cores = 4 batches x 2 query-interleave lanes (j in {0,1}).
Core (b, j) handles q-blocks {(2i+j)*512 : i in 0..3}.  The host hands
each core a *permuted, group-contiguous* x^T whose 512-col groups are
[own-lane blocks | other-lane blocks], so every core runs one identical
instruction stream (SPMD).  Causality reduces to a lane-independent
triangular mask on the own-section diagonal block plus a per-lane
all-0/all-1 scalar on the final other-section chunks.

Key scheduling choices (v2):
  - x is packed group-contiguous on the host (8 KB/partition descriptor
    runs) and fully prefetched into SBUF across the sync/scalar/gpsimd
    DMA rings; group 0 streams per-chunk so the first K matmul starts
    ~1.5 us after the preamble.
  - proj/att interleave is dependency-minimal: proj0, proj4, att0,
    proj1, proj5, att1, ... (att(i) only needs proj groups {0..i, 4..4+i}).
  - softmax denominators: groups 0..2 fold all the way to one tile
    (extra tree adds on gpsimd) -> single sums-matmul; the last group
    keeps the progressive quad scheme so its epilogue chain stays short.
  - pad-masking and pair/quad folds run on gpsimd, keeping DVE free for
    block folds / diagonal masks / PSUM copies.
"""

import sys

sys.path.insert(0, "/opt/trn_rl_repo")

import numpy as np
import ml_dtypes

import concourse.mybir as mybir
import concourse.tile as tile
from concourse import bacc
from concourse.alu_op_type import AluOpType
from concourse.masks import make_identity
from concourse.bass_utils import run_bass_kernel_spmd

B, T, C, H = 4, 4096, 1024, 128
NCORES = 8
QG = 512                      # q-group width
NG = 4                        # q-groups per core
CB = C // 128                 # 8 contraction chunks
TGRP = T // QG                # 8 column groups of x^T
SCALE = float(H) ** -0.5
GPSIMD_FOLDS = False          # gpsimd elementwise is Q7 software: ~25x slower
                              # than DVE ([128,1024] tensor_scalar ~13us) - never route
                              # fold/mask work there

BF16 = mybir.dt.bfloat16
F32 = mybir.dt.float32
NPBF16 = ml_dtypes.bfloat16


def _build_program():
    nc = bacc.Bacc("TRN2", target_bir_lowering=False, debug=False)

    # group-contiguous packed x^T: group g at cols [g*CB*QG, (g+1)*CB*QG),
    # laid out chunk-major: [p, g, c, t]
    xt = nc.dram_tensor("xt", [128, TGRP * CB * QG], BF16, kind="ExternalInput").ap()
    wk = nc.dram_tensor("wk", [128, CB * H], BF16, kind="ExternalInput").ap()
    wq = nc.dram_tensor("wq", [128, CB * H], BF16, kind="ExternalInput").ap()
    wv = nc.dram_tensor("wv", [128, CB * H], BF16, kind="ExternalInput").ap()
    msk = nc.dram_tensor("msk", [128, 4 * QG], BF16, kind="ExternalInput").ap()
    pad = nc.dram_tensor("pad", [128, 1], F32, kind="ExternalInput").ap()
    outT = nc.dram_tensor("outT", [H, NG * QG], BF16, kind="ExternalOutput").ap()

    GW = CB * QG              # 4096 cols per x group

    with tile.TileContext(nc) as tc:
        with (
            tc.tile_pool(name="const", bufs=1) as constp,
            tc.tile_pool(name="kvq", bufs=1) as kvqp,
            tc.tile_pool(name="vtb", bufs=2) as vtbp,
            tc.tile_pool(name="attb", bufs=4) as attp,
            tc.tile_pool(name="foldb", bufs=6) as foldp,
            tc.tile_pool(name="epi", bufs=2) as epip,
            tc.tile_pool(name="pp", bufs=2, space="PSUM") as ppool,
            tc.tile_pool(name="ap", bufs=1, space="PSUM") as apool,
        ):
            # --- persistent SBUF tensors ---
            wks = constp.tile([128, CB * H], BF16, tag="wks")
            wqs = constp.tile([128, CB * H], BF16, tag="wqs")
            wvs = constp.tile([128, CB * H], BF16, tag="wvs")
            masks = constp.tile([128, 4 * QG], BF16, tag="masks")
            padv = constp.tile([128, 1], F32, tag="padv")
            ident = constp.tile([128, 128], BF16, tag="ident")
            XA = constp.tile([128, TGRP * GW], BF16, tag="XA")

            KT = kvqp.tile([128, T], BF16, tag="KT")
            VV = kvqp.tile([128, T], BF16, tag="VV")
            QT = kvqp.tile([128, NG * QG], BF16, tag="QT")
            ones = kvqp.tile([128, 128], BF16, tag="ones")

            # --- input DMA prefetch ---------------------------------------
            # All 16 SDMA engines saturate for ~22us moving x (8 MB), and
            # they round-robin fairly across rings - so both HWDGE rings
            # (scalar, sync) are fed in strict consumption-priority order,
            # interleaved so the two FIFO heads track the same priority
            # frontier: wk, g0, wv|wq, g4, masks, g1, g5, g2, g6, g3, g7.
            # Groups after g0 are split in halves across the rings.
            # both rings lead with half the K weights, then alternate x
            # chunks, so the first K matmul's dependencies land in ~1us
            nc.sync.dma_start(wks[:, :CB * H // 2], wk[:, :CB * H // 2])
            nc.scalar.dma_start(wks[:, CB * H // 2:], wk[:, CB * H // 2:])
            for ci in range(0, CB, 2):
                nc.sync.dma_start(
                    XA[:, ci * QG:(ci + 1) * QG], xt[:, ci * QG:(ci + 1) * QG]
                )
            for ci in range(1, CB, 2):
                nc.scalar.dma_start(
                    XA[:, ci * QG:(ci + 1) * QG], xt[:, ci * QG:(ci + 1) * QG]
                )
            HG = GW // 2
            def xgrp(g, ring_a, ring_b):
                ring_a.dma_start(
                    XA[:, g * GW:g * GW + HG], xt[:, g * GW:g * GW + HG]
                )
                ring_b.dma_start(
                    XA[:, g * GW + HG:(g + 1) * GW], xt[:, g * GW + HG:(g + 1) * GW]
                )
            nc.sync.dma_start(wvs, wv)
            nc.scalar.dma_start(wqs, wq)
            xgrp(4, nc.sync, nc.scalar)
            nc.sync.dma_start(masks, msk)
            nc.scalar.dma_start(padv, pad)
            for g in (1, 5, 2, 6, 3, 7):
                xgrp(g, nc.scalar, nc.sync)
            nc.vector.memset(ones, 1.0)
            make_identity(nc, ident)

            pend_tr = []          # deferred (vtt, tg) transpose work

            def do_transposes():
                while pend_tr:
                    vtt, tg = pend_tr.pop(0)
                    tps = ppool.tile([128, QG], BF16, tag="pps")
                    for tb in range(QG // 128):
                        nc.tensor.transpose(
                            tps[:, tb * 128:(tb + 1) * 128],
                            vtt[:, tb * 128:(tb + 1) * 128],
                            ident,
                        )
                    nc.vector.tensor_copy(VV[:, tg * QG:(tg + 1) * QG], tps)

            def proj_gen(tg, with_q, interleave=False):
                """Projection emitted as a generator: the K and V matmul
                loops yield per-MM so the attention tile loop can feed them
                into its PE slack; the DVE copies / transposes / Q stay in
                the drain (attention's DVE and ACT are the gated engines).
                interleave=True pairs K and V per chunk so the DMA-paced
                startup groups do two matmuls per arriving x chunk."""
                xg = XA[:, tg * GW:(tg + 1) * GW]
                kps = ppool.tile([128, QG], F32, tag="pps")
                if interleave:
                    vps = ppool.tile([128, QG], F32, tag="pps")
                    for ci in range(CB):
                        nc.tensor.matmul(
                            kps,
                            lhsT=wks[:, ci * H:(ci + 1) * H],
                            rhs=xg[:, ci * QG:(ci + 1) * QG],
                            start=(ci == 0),
                            stop=(ci == CB - 1),
                        )
                        nc.tensor.matmul(
                            vps,
                            lhsT=wvs[:, ci * H:(ci + 1) * H],
                            rhs=xg[:, ci * QG:(ci + 1) * QG],
                            start=(ci == 0),
                            stop=(ci == CB - 1),
                        )
                        yield
                else:
                    for ci in range(CB):
                        nc.tensor.matmul(
                            kps,
                            lhsT=wks[:, ci * H:(ci + 1) * H],
                            rhs=xg[:, ci * QG:(ci + 1) * QG],
                            start=(ci == 0),
                            stop=(ci == CB - 1),
                        )
                        yield
                    vps = ppool.tile([128, QG], F32, tag="pps")
                    for ci in range(CB):
                        nc.tensor.matmul(
                            vps,
                            lhsT=wvs[:, ci * H:(ci + 1) * H],
                            rhs=xg[:, ci * QG:(ci + 1) * QG],
                            start=(ci == 0),
                            stop=(ci == CB - 1),
                        )
                        yield
                nc.vector.tensor_copy(KT[:, tg * QG:(tg + 1) * QG], kps)
                do_transposes()    # previous group's V transposes (inputs ready)
                vtt = vtbp.tile([128, QG], BF16, tag="vtt")
                nc.vector.tensor_copy(vtt, vps)
                pend_tr.append((vtt, tg))
                if with_q:
                    qps = ppool.tile([128, QG], F32, tag="pps")
                    for ci in range(CB):
                        nc.tensor.matmul(
                            qps,
                            lhsT=wqs[:, ci * H:(ci + 1) * H],
                            rhs=xg[:, ci * QG:(ci + 1) * QG],
                            start=(ci == 0),
                            stop=(ci == CB - 1),
                        )
                    nc.vector.tensor_copy(QT[:, tg * QG:(tg + 1) * QG], qps)

            def qgen(tg):
                """Q projection alone - hoisted ahead of its group's K/V so
                the following attention group can start before K/V land."""
                xg = XA[:, tg * GW:(tg + 1) * GW]
                qps = ppool.tile([128, QG], F32, tag="pps")
                for ci in range(CB):
                    nc.tensor.matmul(
                        qps,
                        lhsT=wqs[:, ci * H:(ci + 1) * H],
                        rhs=xg[:, ci * QG:(ci + 1) * QG],
                        start=(ci == 0),
                        stop=(ci == CB - 1),
                    )
                    yield
                nc.vector.tensor_copy(QT[:, tg * QG:(tg + 1) * QG], qps)

            def kvgen(tg):
                """K/V projection without Q - the JIT-streamed remainder."""
                xg = XA[:, tg * GW:(tg + 1) * GW]
                kps = ppool.tile([128, QG], F32, tag="pps")
                for ci in range(CB):
                    nc.tensor.matmul(
                        kps,
                        lhsT=wks[:, ci * H:(ci + 1) * H],
                        rhs=xg[:, ci * QG:(ci + 1) * QG],
                        start=(ci == 0),
                        stop=(ci == CB - 1),
                    )
                    yield
                vps = ppool.tile([128, QG], F32, tag="pps")
                for ci in range(CB):
                    nc.tensor.matmul(
                        vps,
                        lhsT=wvs[:, ci * H:(ci + 1) * H],
                        rhs=xg[:, ci * QG:(ci + 1) * QG],
                        start=(ci == 0),
                        stop=(ci == CB - 1),
                    )
                    yield
                nc.vector.tensor_copy(KT[:, tg * QG:(tg + 1) * QG], kps)
                do_transposes()
                vtt = vtbp.tile([128, QG], BF16, tag="vtt")
                nc.vector.tensor_copy(vtt, vps)
                pend_tr.append((vtt, tg))

            feeders = []
            jit_drains = {}       # att tile index -> generator to drain first

            def feed(n):
                while n > 0 and feeders:
                    try:
                        next(feeders[0])
                        n -= 1
                    except StopIteration:
                        feeders.pop(0)

            def drain_gen(g):
                # exhaust the FIFO up to and including generator g
                while g in feeders:
                    try:
                        next(feeders[0])
                    except StopIteration:
                        feeders.pop(0)

            def proj(tg, with_q):
                for _ in proj_gen(tg, with_q):
                    pass

            def att(i):
                do_transposes()    # flush V transposes the group reads
                qg = QT[:, i * QG:(i + 1) * QG]
                # consecutive att groups are separated by two proj groups,
                # so the previous group's epilogue has long released these
                otps = apool.tile([128, QG], F32, tag="otps", bufs=1)
                smps = apool.tile([128, QG], F32, tag="smps", bufs=1)
                ntiles = 2 * (i + 1)
                # tiles: chunk base; mask kind (None | diag-offset | 'pad')
                tiles = []
                for sec in range(2):
                    for tp in range(ntiles):
                        mt = tp - (ntiles - 2)
                        if mt < 0:
                            mk = None
                        elif sec == 0:
                            mk = mt * 2 * QG
                        else:
                            mk = "pad"
                        tiles.append((16 * sec + 2 * tp, mk))
                ntot = len(tiles)
                sps_t = [None] * ntot
                pt_t = [None] * ntot
                fold_t = [None] * ntot

                HQ = QG // 2

                def emit_s(t):
                    c0, mk = tiles[t]
                    # upper diagonal tile (kv offsets 256..511): fully masked
                    # for q < 256, so only compute the upper q-half; the mask
                    # multiply zeroes the stale lower half before use.  NOT
                    # for group 0: its tile 1 is the psum slot's first-ever
                    # use, and exp(uninitialized psum) can be inf -> inf*0
                    # mask -> NaN.  Later groups reuse slots already holding
                    # finite S values, so their stale halves exp to finite.
                    sh = HQ if (mk == 2 * QG and i > 0) else 0
                    sps = apool.tile([128, 2 * QG], F32, tag="sps", bufs=2)
                    for h in range(2):
                        nc.tensor.matmul(
                            sps[:, h * QG + sh:(h + 1) * QG],
                            lhsT=KT[:, (c0 + h) * 128:(c0 + h + 1) * 128],
                            rhs=qg[:, sh:],
                            start=True,
                            stop=True,
                        )
                    sps_t[t] = sps

                def emit_exp_mask_fold(t):
                    _, mk = tiles[t]
                    pt = attp.tile([128, 2 * QG], BF16, tag="pt")
                    # pad tiles: fold the per-lane kill into the exp bias
                    # (lane 0: -1e9 -> exp underflows to exact 0; lane 1: 0)
                    nc.scalar.activation(
                        pt, sps_t[t], mybir.ActivationFunctionType.Exp,
                        scale=SCALE,
                        bias=padv if mk == "pad" else 0.0,
                    )
                    sps_t[t] = None
                    if mk is not None and mk != "pad":
                        nc.vector.tensor_tensor(
                            pt, pt, masks[:, mk:mk + 2 * QG], op=AluOpType.mult
                        )
                    pt_t[t] = pt
                    if last and t == ntot - 1:
                        return    # final tile sums straight from pt halves
                    fold = foldp.tile([128, QG], BF16, tag="fold")
                    nc.vector.tensor_tensor(
                        fold, pt[:, 0:QG], pt[:, QG:2 * QG], op=AluOpType.add
                    )
                    fold_t[t] = fold

                def emit_pv(t):
                    c0, mk = tiles[t]
                    sh = HQ if (mk == 2 * QG and i > 0) else 0  # match emit_s
                    for h in range(2):
                        c = c0 + h
                        nc.tensor.matmul(
                            otps[:, sh:],
                            lhsT=VV[:, c * 128:(c + 1) * 128],
                            rhs=pt_t[t][:, h * QG + sh:(h + 1) * QG],
                            start=(t == 0 and h == 0),
                            stop=(t == ntot - 1 and h == 1),
                        )
                    pt_t[t] = None

                feng = nc.gpsimd if GPSIMD_FOLDS else nc.vector

                ffs = []        # pair-folds awaiting a quad partner
                qmm = []        # quad-folds awaiting their sums matmul
                nsum = [0]
                deep = i < NG - 1      # groups 0..2: fold to one sums-matmul
                last = i == NG - 1     # final group: short epilogue chain
                NSUM = 1 if deep else (ntot // 4 - 1) + 4

                def emit_pair(t):
                    # pair-fold tiles (t-1, t); every 2nd pair quad-folds
                    ff = foldp.tile([128, QG], BF16, tag="ffold", bufs=4)
                    feng.tensor_tensor(
                        ff, fold_t[t - 1], fold_t[t], op=AluOpType.add
                    )
                    fold_t[t - 1] = fold_t[t] = None
                    ffs.append(ff)
                    if len(ffs) == 2:
                        fff = foldp.tile([128, QG], BF16, tag="fff", bufs=3)
                        feng.tensor_tensor(
                            fff, ffs[0], ffs[1], op=AluOpType.add
                        )
                        ffs.clear()
                        qmm.append(fff)

                def flush_sums(final=False):
                    if deep:
                        # fold quads into one running tile; single sums-matmul
                        while len(qmm) >= 2:
                            a = qmm.pop(0)
                            b = qmm.pop(0)
                            acc = foldp.tile([128, QG], BF16, tag="fff", bufs=3)
                            feng.tensor_tensor(acc, a, b, op=AluOpType.add)
                            qmm.insert(0, acc)
                        if final:
                            assert len(qmm) == 1
                            fff = qmm.pop(0)
                            nc.tensor.matmul(
                                smps, lhsT=ones, rhs=fff, start=True, stop=True,
                            )
                        return
                    while qmm:
                        fff = qmm.pop(0)
                        q = nsum[0]
                        nsum[0] += 1
                        nc.tensor.matmul(
                            smps, lhsT=ones, rhs=fff,
                            start=(q == 0), stop=(q == NSUM - 1),
                        )

                emit_s(0)
                emit_exp_mask_fold(0)
                for t in range(ntot):
                    if (t + 1) in jit_drains:
                        # JIT-streamed K/V must land before the next tile's
                        # S/PV read it; flush the pending V transposes too
                        drain_gen(jit_drains.pop(t + 1))
                        do_transposes()
                    if t + 1 < ntot:
                        emit_s(t + 1)
                        emit_exp_mask_fold(t + 1)
                    if last and t == ntot - 1:
                        # promote pair(n-4,n-3) and fold(n-2) straight to
                        # sums-matmuls before the last PV, so the epilogue
                        # chain after exp(n-1) is just sums -> recip
                        ff = foldp.tile([128, QG], BF16, tag="ffold", bufs=4)
                        nc.vector.tensor_tensor(
                            ff, fold_t[t - 3], fold_t[t - 2], op=AluOpType.add
                        )
                        fold_t[t - 3] = fold_t[t - 2] = None
                        qmm.append(ff)
                        qmm.append(fold_t[t - 1])
                        fold_t[t - 1] = None
                        # the final tile's unfolded pt halves go straight to
                        # sums-matmuls BEFORE its PV, so the reciprocals run
                        # during the last PV instead of after it
                        qmm.append(pt_t[t][:, 0:QG])
                        qmm.append(pt_t[t][:, QG:2 * QG])
                        flush_sums()
                    emit_pv(t)
                    feed(1)    # one deferred proj MM into this tile's PE slack
                    if t % 2 == 1 and t >= 3 and not (last and t == ntot - 1):
                        flush_sums()          # quads trail two more tiles
                        emit_pair(t - 2)      # pairs trail two tiles
                if not last:
                    emit_pair(ntot - 1)
                    flush_sums(final=True)
                HQ = QG // 2
                for hh in range(2):
                    sl = slice(hh * HQ, (hh + 1) * HQ)
                    rb = epip.tile([128, HQ], F32, tag="rb", bufs=2)
                    nc.vector.reciprocal_approx_fast(rb, smps[:, sl])
                    ot = epip.tile([128, HQ], BF16, tag="ot", bufs=2)
                    nc.vector.tensor_tensor(ot, otps[:, sl], rb,
                                            op=AluOpType.mult)
                    dst = outT[:, i * QG + hh * HQ:i * QG + (hh + 1) * HQ]
                    # alternate rings so the two ~600ns DMA triggers run in
                    # parallel; the final group uses the vector/gpsimd rings
                    # (same-engine ordering after the mult - no sem hop)
                    (nc.sync if hh == 0 else nc.scalar).dma_start(dst, ot)

            # just-in-time projection streaming: att(i) only needs Q(i) up
            # front; its own K/V aren't read until the diagonal tiles (2i)
            # and the other-lane K/V until tile 2(i+1)+2i, so groups 2,3
            # stream their K/V into the preceding atts' exp-gated PE slack
            proj(0, with_q=True)
            proj(4, with_q=False)
            qg1, kv1, kv5 = qgen(1), kvgen(1), kvgen(5)
            feeders[:] = [qg1, kv1, kv5]
            att(0)
            drain_gen(kv1)           # att1 reads K1/V1 at its diag tile 2
            jit_drains[6] = kv5      # att1 other tail
            qg2, kv2, kv6 = qgen(2), kvgen(2), kvgen(6)
            feeders.extend([qg2, kv2, kv6])
            att(1)
            drain_gen(qg2)
            qg3, kv3, kv7 = qgen(3), kvgen(3), kvgen(7)
            feeders.extend([qg3, kv3, kv7])
            jit_drains[4] = kv2      # att2 own diag
            jit_drains[10] = kv6     # att2 other tail
            att(2)
            drain_gen(qg3)
            jit_drains[6] = kv3      # att3 own diag
            jit_drains[14] = kv7     # att3 other tail
            att(3)

    if not nc.is_finalized():
        nc.finalize()
    return nc


_NC_CACHE = None


def _get_program():
    global _NC_CACHE
    if _NC_CACHE is None:
        _NC_CACHE = _build_program()
    return _NC_CACHE


def _make_masks() -> np.ndarray:
    """Triangular masks [128, 2048] for the 4 chunks of the own-section
    diagonal block (chunk c masked where 128*c + kv > q), lane-independent."""
    out = np.empty((128, 4 * QG), np.float32)
    kv = np.arange(128)[:, None]
    q = np.arange(QG)[None, :]
    for c in range(4):
        out[:, c * QG:(c + 1) * QG] = (128 * c + kv <= q)
    return out.astype(NPBF16)


def _pack_weight(w: np.ndarray) -> np.ndarray:
    # [C, H] -> [128, CB*H] with chunk-major cols: out[p, c*H+h] = w[c*128+p, h]
    return np.ascontiguousarray(
        w.reshape(CB, 128, H).transpose(1, 0, 2).reshape(128, CB * H)
    ).astype(NPBF16)


def _run(inputs: dict, trace: bool = False, trace_kwargs: dict | None = None):
    x = np.asarray(inputs["x"], np.float32)
    Wk = np.asarray(inputs["Wk"], np.float32)
    Wq = np.asarray(inputs["Wq"], np.float32)
    Wv = np.asarray(inputs["Wv"], np.float32)

    nc = _get_program()

    wk16 = _pack_weight(Wk)
    wq16 = _pack_weight(Wq)
    wv16 = _pack_weight(Wv)
    msk = _make_masks()
    # exp-bias pad kill: lane 0 gets -1e9 (exp -> 0), lane 1 gets 0 (keep)
    pads = [np.full((128, 1), (j - 1) * 1e9, np.float32) for j in range(2)]

    in_maps = []
    for b in range(B):
        # [CB, 128, TGRP, QG]: xtr[c, p, blk, t] = x[b, blk*QG+t, c*128+p]
        xtr = np.ascontiguousarray(x[b].T).astype(NPBF16).reshape(
            CB, 128, TGRP, QG
        )
        for j in range(2):
            order = [2 * i + j for i in range(NG)] + [
                2 * i + 1 - j for i in range(NG)
            ]
            # pack group-contiguous chunk-major: xp[p, g, c, t]
            xp = np.ascontiguousarray(
                xtr[:, :, order, :].transpose(1, 2, 0, 3)
            ).reshape(128, TGRP * CB * QG)
            in_maps.append(
                {
                    "xt": xp,
                    "wk": wk16,
                    "wq": wq16,
                    "wv": wv16,
                    "msk": msk,
                    "pad": pads[j],
                }
            )

    res = run_bass_kernel_spmd(
        nc,
        in_maps,
        core_ids=list(range(NCORES)),
        trace=trace,
        **(trace_kwargs or {}),
    )

    out = np.empty((B, T, H), np.float32)
    for core in range(NCORES):
        b, j = divmod(core, 2)
        oT = np.asarray(res.results[core]["outT"], np.float32)  # [H, NG*QG]
        for i in range(NG):
            g = (2 * i + j) * QG
            out[b, g:g + QG, :] = oT[:, i * QG:(i + 1) * QG].T
    return out, res


def kernel(**inputs) -> np.ndarray:
    out, _ = _run(inputs, trace=False)
    return out
